# revision 45
# baseline (speedup 1.0000x reference)
"""APKDA loss (pool+normalize -> SmoothAP) as two distributed Bass launches on
8 TRN2 NeuronCores.

Math restructuring vs the reference (same as the earlier baseline):
  - Only the diagonal class-blocks of sim_all_rk are used, so per query q we
    need rank sums only over its 16 same-class columns j:
        r_all[q,j] = 1 + sum_k relu(S[q,k] - S[q,j])   (k over all 512)
        r_pos[q,j] = 1 + sum_k relu(Sg[q,k] - Sg[q,j]) (k over the 16-group)
  - L2-normalizing the hw-sum equals normalizing the hw-mean.
  - Column order of S is irrelevant; each core's keys are rotated so its own
    64 columns sit first.

Precision: inputs are cast to fp8_e4m3 on the host (4x fewer HBM bytes; the
errors average out over the 49-wide pooling and 512-d normalized dot
products; measured end-to-end rel-err ~1e-4 vs the f32 reference, tolerance
2e-2).  f is also shipped as fp8 in phase 2; biases stay f32.

Phase 1 (memory-bound): core m owns batch rows 32m..32m+31 of both branches.
  - Branch A (outputs) is pooled on the PE: host lays the shard out as 13
    h-plane tiles [128 part=(hh,b), 512 c] fp8; 13 accumulating matmuls
    against a one-hot [128, 32] "eye" (eye[p, p%32]=1) give
    psum[b, c] = sum_h x[b, c, h] exactly (f32 accumulate).
  - Branch B (targets) is reduced on the DVE from the classic
    [128 part=(g,b), (c h)] layout in 4 c-chunks (TensorReduce is ~1.1ns/col
    regardless of dtype; one branch = 7.1us; PE's 13 matmuls ~5.6us + the
    psum->sbuf copy are the other pole -- balanced).
  - The logical-core pair shares ~400GB/s of SDMA/HBM, so the 1.7MB shard
    lands over ~9us however it is laid out; sync carries xa, scalar xb,
    chunked so both engines chase the stream and the last chunks are small.
    Engine DMA issue costs ~0.6-1.1us per 128-descriptor dma_start, and
    per-partition runs below ~4KB lose DMA efficiency -- so few, fat,
    full-width chunks.  Out: pooled sums, bf16, split so the tail DMA is
    tiny.

Phase 2: host normalizes f, rotates keys per core, computes the own-class
Gram and rank biases in f32 (tiny), and ships fp8 fT (rhs [128p, 4x512] =
2KB/partition single DMA) + fp8 ccin (own queries duplicated on both psum
partition halves) + f32 biases.  PE computes the S slice (8 fp8 matmuls,
2 psum banks x 4 d-chunks); ACT/DVE stage S to bf16 and run the 8 big rank
ops (~730ns each, split 4/4) with accum_out; the 8 r_pos ops run early on
ACT straight from the host Gram.  Host applies +1/division/total.

Collectives/single-launch variants measured earlier: in-kernel AllGather
133.7us (nrt cc latency floor ~100us); remote-dma exchange requires
cross-core waits that absorb the profiler's multi-ms launch skew into core
0's measured time.  So the f exchange goes through the host (two NEFF
launches; each carries ~6us preamble + ~7us NEFF postamble of fixed cost).
"""

import numpy as np
import ml_dtypes

import concourse.bass as bass
import concourse.bacc as bacc
import concourse.mybir as mybir
from concourse.bass_utils import run_bass_kernel_spmd

F32 = mybir.dt.float32
BF16 = mybir.dt.bfloat16
F8 = mybir.dt.float8e4
NP_F8 = ml_dtypes.float8_e4m3
NCORES = 8
BATCH, FEAT, HW = 256, 512, 49
BPC = BATCH // NCORES          # 32 batch rows per branch per core
GROUP, B2 = 16, 512

XB_CHUNKS = [32, 40, 40, 16]   # branch-B c-units (x49 cols) per DVE chunk
XA_CHUNKS = [2, 4, 4, 2]       # branch-A h-plane tiles per PE chunk (12 full
                               # tiles; plane 48 ships separately as [32,512])


def _hoist_to_preamble(nc, head, early=True):
    """Move the given BassInstructions to the head of their engine's
    stream.  early=True inserts right after the engine's first DRAIN --
    BEFORE the ctor barrier that serializes everything behind gpsimd's
    ~6us library-load, so the transfers run during the dead preamble
    window.  early=False is the old post-preamble_end hoist."""
    entry = nc.main_func.blocks[0]
    lst = entry.instructions
    for bi in reversed(head):
        inst = bi.ins
        lst.remove(inst)
        if early:
            idx = next(i for i, x in enumerate(lst)
                       if getattr(x, "engine", None) == inst.engine
                       and isinstance(x, mybir.InstDrain))
            lst.insert(idx + 1, inst)
        else:
            pe_marker = nc.engines[inst.engine].preamble_end
            lst.insert(lst.index(pe_marker) + 1, inst)


def build_phase1(dbg=None):
    """fp8 pooling: PE eye-matmul for branch A, DVE reduce for branch B."""
    nc = bacc.Bacc("TRN2", target_bir_lowering=False, debug=False,
                   num_devices=NCORES)
    AX = mybir.AxisListType
    # xa: branch A h-plane tiles; [128, 6656] but only [0:32] of the last
    # 512 cols is read (plane 48).  xb: branch B [(g,b), (c h)] fp8.
    xa_d = nc.dram_tensor("xa", [128, 6656], F8, kind="ExternalInput")
    xb_d = nc.dram_tensor("xb", [128, 6272], F8, kind="ExternalInput")
    eye_d = nc.dram_tensor("eye", [128, 32], F8, kind="ExternalInput")
    pa_d = nc.dram_tensor("pa", [32, 512], BF16, kind="ExternalOutput")
    pt_d = nc.dram_tensor("pt", [128, 128], BF16, kind="ExternalOutput")

    xa = nc.alloc_sbuf_tensor("xa_s", [128, 6656], F8)
    xb = nc.alloc_sbuf_tensor("xb_s", [128, 6272], F8)
    eye = nc.alloc_sbuf_tensor("eye_s", [128, 32], F8)
    pa = nc.alloc_sbuf_tensor("pa_s", [32, 512], BF16)
    pt = nc.alloc_sbuf_tensor("pt_s", [128, 128], BF16)
    ps = nc.alloc_psum_tensor("ps", [32, 512], F32)

    sEYE = nc.alloc_semaphore("sEYE")
    sB = [nc.alloc_semaphore(f"sB{i}") for i in range(len(XB_CHUNKS))]
    sA = [nc.alloc_semaphore(f"sA{i}") for i in range(len(XA_CHUNKS))]
    sAL = nc.alloc_semaphore("sAL")
    sPE = nc.alloc_semaphore("sPE")
    sCP = nc.alloc_semaphore("sCP")
    sRED = nc.alloc_semaphore("sRED")
    sOUT = nc.alloc_semaphore("sOUT")
    sems = [sEYE] + sB + sA + [sAL, sPE, sCP, sRED, sOUT]
    nums = sorted(s.num for s in sems)
    assert nums == list(range(nums[0], nums[0] + len(sems))), nums
    sem_range = range(nums[0], nums[0] + len(sems))

    head = []
    # --- chunked DMAs on both queues: the logical-core pair shares
    # ~400GB/s of SDMA/HBM, so the shard lands over ~8.5us no matter the
    # layout; chunking lets PE/DVE chase the stream and finish ~0.5us
    # after the last byte instead of starting then. sync=xa, scalar=xb.
    # one stream per queue: cross-queue interleaving makes each stream's
    # next chunk wait behind the other stream's chunks (measured worse)
    t0 = 0
    for i, nt in enumerate(XA_CHUNKS):
        head.append(nc.sync.dma_start(
            xa.ap()[:, 512 * t0:512 * (t0 + nt)],
            xa_d.ap()[:, 512 * t0:512 * (t0 + nt)]).then_inc(sA[i], 16))
        if i == 0:
            head.append(nc.sync.dma_start(eye.ap(), eye_d.ap()
                                          ).then_inc(sEYE, 16))
        t0 += nt
    head.append(nc.sync.dma_start(
        xa.ap()[0:32, 6144:6656], xa_d.ap()[0:32, 6144:6656]
    ).then_inc(sAL, 16))
    c0 = 0
    for i, w in enumerate(XB_CHUNKS):
        head.append(nc.scalar.dma_start(
            xb.ap()[:, 49 * c0:49 * (c0 + w)],
            xb_d.ap()[:, 49 * c0:49 * (c0 + w)]).then_inc(sB[i], 16))
        c0 += w

    # --- tensor: 12 full accumulating matmuls + the K=32 plane-48 one ----
    nc.tensor.wait_ge(sEYE, 16)
    t0 = 0
    for i, nt in enumerate(XA_CHUNKS):
        nc.tensor.wait_ge(sA[i], 16)
        for t in range(t0, t0 + nt):
            nc.tensor.matmul(ps.ap(), eye.ap(),
                             xa.ap()[:, 512 * t:512 * (t + 1)],
                             start=(t == 0), stop=False)
        t0 += nt
    nc.tensor.wait_ge(sAL, 16)
    mm = nc.tensor.matmul(ps.ap(), eye.ap()[0:32, :],
                          xa.ap()[0:32, 6144:6656], start=False, stop=True)
    mm.then_inc(sPE, 1)

    # --- scalar (ACT): psum -> bf16 pa, then out DMA ---------------------
    nc.scalar.wait_ge(sPE, 1)
    with nc.allow_low_precision("pooled sums, single bf16 round"):
        nc.scalar.copy(pa.ap(), ps.ap()).then_inc(sCP, 1)
    nc.scalar.wait_ge(sCP, 1)
    nc.scalar.dma_start(pa_d.ap(), pa.ap()).then_inc(sOUT, 16)

    # --- vector (DVE): 4 reduces chasing the xb chunks -------------------
    c0 = 0
    nw = len(XB_CHUNKS)
    for i, w in enumerate(XB_CHUNKS):
        nc.vector.wait_ge(sB[i], 16)
        with nc.allow_low_precision("f32 accumulate, single bf16 round"):
            red = nc.vector.reduce_sum(
                pt.ap()[:, c0:c0 + w],
                xb.ap()[:, 49 * c0:49 * (c0 + w)].rearrange(
                    "p (c h) -> p c h", h=HW),
                axis=AX.X)
        if i >= nw - 2:
            red.then_inc(sRED, 1)
        c0 += w
    # split pt out: everything but the last chunk ships early, so the
    # final out DMA (and its HBM completion) is tiny
    c1 = sum(XB_CHUNKS[:-1])
    nc.sync.wait_ge(sRED, 1)
    nc.sync.dma_start(pt_d.ap()[:, 0:c1], pt.ap()[:, 0:c1]).then_inc(sOUT, 16)
    nc.sync.wait_ge(sRED, 2)
    nc.sync.dma_start(pt_d.ap()[:, c1:128], pt.ap()[:, c1:128]).then_inc(sOUT, 16)

    # --- gpsimd: restore sems, halt --------------------------------------
    nc.gpsimd.wait_ge(sOUT, 48)
    nc.gpsimd.dma_reset(sem_range)
    nc.gpsimd.sem_clear(sem_range)

    _hoist_to_preamble(nc, head)
    nc.compile()
    return nc


def build_phase2(dbg=None):
    """S slice + rank sums from fp8 fT (own cols at keys 0..63).
    in: rhs [128, 2048] fp8 (4 d-blocks side by side, rhs[p, 512g+k] =
    fT[d=128g+p, key k]); ccin [128, 512] fp8 (ccin[p, 128g+64*two+q] =
    f_own[q, d=128g+p], queries duplicated on the two psum halves);
    bias [128, 24] f32 (cols 0-15 SgD, 16-23 B8 = -bias per j-slot);
    out racc [128, 16] f32 (cols 0-7 raw r_all, 8-15 raw r_pos)."""
    nc = bacc.Bacc("TRN2", target_bir_lowering=False, debug=False,
                   num_devices=NCORES)
    AF = mybir.ActivationFunctionType
    ALU = mybir.AluOpType
    rhs_d = nc.dram_tensor("rhs", [128, 2048], F8, kind="ExternalInput")
    ccin_d = nc.dram_tensor("ccin", [128, 512], F8, kind="ExternalInput")
    bias_d = nc.dram_tensor("bias", [128, 24], F32, kind="ExternalInput")
    out_d = nc.dram_tensor("out", [128, 16], F32, kind="ExternalOutput")

    rhs = nc.alloc_sbuf_tensor("rhs_s", [128, 2048], F8)
    ccin = nc.alloc_sbuf_tensor("ccin_s", [128, 512], F8)
    bias = nc.alloc_sbuf_tensor("bias_s", [128, 24], F32)
    Sb = nc.alloc_sbuf_tensor("Sb", [128, 512], BF16)
    scrap_v = nc.alloc_sbuf_tensor("scrap_v", [128, 512], BF16)
    scrap_a = nc.alloc_sbuf_tensor("scrap_a", [128, 512], BF16)
    scrap_s = nc.alloc_sbuf_tensor("scrap_s", [128, 16], F32)
    zeros = nc.alloc_sbuf_tensor("zeros", [128, 512], BF16)
    warm = nc.alloc_sbuf_tensor("warm", [128, 1], F32)
    racc = nc.alloc_sbuf_tensor("racc", [128, 16], F32)
    ps0 = nc.alloc_psum_tensor("ps0", [128, 256], F32)
    ps1 = nc.alloc_psum_tensor("ps1", [128, 256], F32)

    sRHg = [nc.alloc_semaphore(f"sRH{g}") for g in range(4)]
    sCC = nc.alloc_semaphore("sCC")
    sBI = nc.alloc_semaphore("sBI")
    sPE0 = nc.alloc_semaphore("sPE0")
    sPE1 = nc.alloc_semaphore("sPE1")
    sST = nc.alloc_semaphore("sST")
    sRKS = nc.alloc_semaphore("sRKS")
    sRKB = nc.alloc_semaphore("sRKB")
    sRKA = nc.alloc_semaphore("sRKA")
    sOUT = nc.alloc_semaphore("sOUT")
    sems = sRHg + [sCC, sBI, sPE0, sPE1, sST, sRKS, sRKB, sRKA, sOUT]
    nums = sorted(s.num for s in sems)
    assert nums == list(range(nums[0], nums[0] + len(sems))), nums
    sem_range = range(nums[0], nums[0] + len(sems))

    head = []
    head.append(nc.sync.dma_start(
        rhs.ap()[:, 0:1024], rhs_d.ap()[:, 0:1024]).then_inc(sRHg[0], 16))
    head.append(nc.scalar.dma_start(bias.ap(), bias_d.ap()).then_inc(sBI, 16))
    head.append(nc.scalar.dma_start(ccin.ap(), ccin_d.ap()).then_inc(sCC, 16))
    head.append(nc.scalar.dma_start(
        rhs.ap()[:, 1024:2048], rhs_d.ap()[:, 1024:2048]).then_inc(sRHg[2], 16))

    # --- tensor: 8 fp8 matmuls chasing the rhs halves --------------------
    nc.tensor.wait_ge(sCC, 16)
    for g in range(4):
        if g % 2 == 0:
            nc.tensor.wait_ge(sRHg[g], 16)
        mm0 = nc.tensor.matmul(ps0.ap(), ccin.ap()[:, 128 * g:128 * (g + 1)],
                               rhs.ap()[:, 512 * g:512 * g + 256],
                               start=(g == 0), stop=(g == 3))
        mm1 = nc.tensor.matmul(ps1.ap(), ccin.ap()[:, 128 * g:128 * (g + 1)],
                               rhs.ap()[:, 512 * g + 256:512 * (g + 1)],
                               start=(g == 0), stop=(g == 3))
        if g == 3:
            mm0.then_inc(sPE0, 1)
            mm1.then_inc(sPE1, 1)

    # --- scalar (ACT): warm, ps0 stage, 4 big ranks ----------------------
    nc.scalar.wait_ge(sBI, 16)
    nc.scalar.activation(warm.ap(), bias.ap()[:, 0:1], AF.Relu)
    nc.scalar.wait_ge(sPE0, 1)
    with nc.allow_low_precision("psum f32 -> bf16 S"):
        nc.scalar.copy(Sb.ap()[:, 0:256], ps0.ap()).then_inc(sST, 1)
    nc.scalar.wait_ge(sST, 2)
    with nc.allow_low_precision("bf16 rank scrap, f32 accum"):
        for i in range(4, 8):
            nc.scalar.activation(
                scrap_a.ap(), Sb.ap(), AF.Relu,
                bias=bias.ap()[:, 16 + i:17 + i],
                accum_out=racc.ap()[:, i:i + 1]).then_inc(sRKA, 1)

    # --- vector (DVE): early r_pos ranks (cheap 85ns accum reads), ps1
    # stage, 4 big ranks --------------------------------------------------
    nc.vector.memset(zeros.ap(), 0.0)
    nc.vector.wait_ge(sBI, 16)
    for i in range(8):
        nc.vector.scalar_tensor_tensor(
            out=scrap_s.ap(), in0=bias.ap()[:, 0:16],
            scalar=bias.ap()[:, 16 + i:17 + i], in1=zeros.ap()[:, 0:16],
            op0=ALU.add, op1=ALU.max,
            accum_out=racc.ap()[:, 8 + i:9 + i]).then_inc(sRKS, 1)
    nc.vector.wait_ge(sPE1, 1)
    with nc.allow_low_precision("psum f32 -> bf16 S"):
        nc.vector.tensor_copy(Sb.ap()[:, 256:512], ps1.ap()).then_inc(sST, 1)
    nc.vector.wait_ge(sST, 2)
    with nc.allow_low_precision("bf16 rank scrap, f32 accum"):
        for i in range(4):
            nc.vector.scalar_tensor_tensor(
                out=scrap_v.ap(), in0=Sb.ap(),
                scalar=bias.ap()[:, 16 + i:17 + i], in1=zeros.ap(),
                op0=ALU.add, op1=ALU.max,
                accum_out=racc.ap()[:, i:i + 1]).then_inc(sRKB, 1)

    # --- sync: split out DMAs (r_pos half and each engine's big half ship
    # as soon as they are ready, so the last HBM completion starts early) -
    nc.sync.wait_ge(sRKS, 8)
    nc.sync.dma_start(out_d.ap()[:, 8:16], racc.ap()[:, 8:16]).then_inc(sOUT, 16)
    nc.sync.wait_ge(sRKB, 4)
    nc.sync.dma_start(out_d.ap()[:, 0:4], racc.ap()[:, 0:4]).then_inc(sOUT, 16)
    nc.sync.wait_ge(sRKA, 4)
    nc.sync.dma_start(out_d.ap()[:, 4:8], racc.ap()[:, 4:8]).then_inc(sOUT, 16)

    # --- gpsimd: restore sems, halt --------------------------------------
    nc.gpsimd.wait_ge(sOUT, 48)
    nc.gpsimd.dma_reset(sem_range)
    nc.gpsimd.sem_clear(sem_range)

    _hoist_to_preamble(nc, head)
    nc.compile()
    return nc


_NC1 = None
_NC2 = None


def _get_ncs():
    global _NC1, _NC2
    if _NC1 is None:
        _NC1 = build_phase1()
        _NC2 = build_phase2()
    return _NC1, _NC2


# one-hot pooling matrix: eye[p, p%32] = 1
_EYE = np.zeros((128, 32), np.float32)
_EYE[np.arange(128), np.arange(128) % 32] = 1.0
_EYE = _EYE.astype(NP_F8)

# column permutation: branch-ordered [out b, tgt b] -> reference interleaved
# col = 16*(b//8) + 8*branch + b%8
_PERM = np.empty(64, np.int64)
for _b in range(32):
    _PERM[16 * (_b // 8) + (_b % 8)] = _b            # outputs branch
    _PERM[16 * (_b // 8) + 8 + (_b % 8)] = 32 + _b   # targets branch


def make_in_maps1(outputs, targets):
    outputs = np.asarray(outputs, dtype=np.float32).reshape(BATCH, FEAT, HW)
    targets = np.asarray(targets, dtype=np.float32).reshape(BATCH, FEAT, HW)
    o8 = outputs.astype(NP_F8)
    t8 = targets.astype(NP_F8)
    maps = []
    for m in range(NCORES):
        o = o8[m * BPC:(m + 1) * BPC]                  # [32, 512, 49]
        t = t8[m * BPC:(m + 1) * BPC]
        # branch A: h-plane tiles.  xa[32*hh+b, 512*t+c] = o[b, c, 4t+hh]
        ot = o.transpose(2, 0, 1)                      # [49, 32, 512]
        xa = np.zeros((128, 6656), NP_F8)
        xa[:, 0:6144] = (ot[0:48].reshape(12, 4, BPC, FEAT)
                         .transpose(0, 1, 2, 3)        # [t, hh, b, c]
                         .reshape(12, 128, FEAT)
                         .transpose(1, 0, 2)           # [p, t, c]
                         .reshape(128, 6144))
        xa[0:32, 6144:6656] = ot[48]                   # plane 48
        # branch B: [(g,b), (c h)]: xb[32g+b] = t[b, 128g:128g+128, :].flat
        xb = (t.reshape(BPC, 4, 128, HW)
              .transpose(1, 0, 2, 3)                   # [g, b, c_local, h]
              .reshape(128, 6272))
        maps.append({"xa": np.ascontiguousarray(xa),
                     "xb": np.ascontiguousarray(xb), "eye": _EYE})
    return maps


def make_in_maps2(results1):
    """pooled sums -> normalized f, per-core rotated fp8 fT + biases."""
    blocks = []   # per core: f rows [64, 512] f32 in reference order
    for m in range(NCORES):
        pa = results1[m]["pa"].astype(np.float32)      # [32, 512] b-major
        pt = results1[m]["pt"].astype(np.float32)      # [128(g,b), 128]
        vt = np.concatenate([pt[32 * g:32 * (g + 1), :] for g in range(4)],
                            axis=1)                    # [32, 512]
        f = np.concatenate([pa, vt], axis=0)           # [64, 512]
        f /= np.linalg.norm(f, axis=1, keepdims=True)
        blocks.append(f[_PERM, :])                     # reference order
    f8 = [b.astype(NP_F8) for b in blocks]
    f8f = [b.astype(np.float32) for b in f8]           # fp8-quantized f32
    maps = []
    for m in range(NCORES):
        # keys rotated: own 64 first
        rot = np.concatenate([f8f[(m + j) % NCORES] for j in range(NCORES)],
                             axis=0)                   # [512 keys, 512 d]
        rhs = np.ascontiguousarray(
            rot.T.reshape(4, 128, 512).transpose(1, 0, 2).reshape(128, 2048)
        ).astype(NP_F8)                                # [p, 512g+k]
        own = f8f[m]                                   # [64 q, 512 d]
        ccin = np.empty((128, 512), np.float32)
        for g in range(4):
            blk = own[:, 128 * g:128 * (g + 1)].T      # [128 d, 64 q]
            ccin[:, 128 * g + 0:128 * g + 64] = blk
            ccin[:, 128 * g + 64:128 * g + 128] = blk
        sg = own @ own.T                               # [64, 64] f32
        # SgD[p, j] = Sg[qi, 16*(qi//16)+j], qi = p % 64
        qi = np.arange(64)
        base = (qi // 16) * 16
        sgd64 = sg[qi[:, None], base[:, None] + np.arange(16)[None]]  # [64,16]
        sgd = np.concatenate([sgd64, sgd64], axis=0)   # [128, 16]
        b8 = np.empty((128, 8), np.float32)
        b8[0:64] = -sgd64[:, 0:8]
        b8[64:128] = -sgd64[:, 8:16]
        biasm = np.concatenate([sgd, b8], axis=1)      # [128, 24]
        maps.append({"rhs": rhs, "ccin": ccin.astype(NP_F8),
                     "bias": np.ascontiguousarray(biasm)})
    return maps


def finish(results2):
    total = 0.0
    for m in range(NCORES):
        racc = results2[m]["out"].astype(np.float64)   # [128, 16]
        total += ((1.0 + racc[:, 8:16]) / (1.0 + racc[:, 0:8])).sum()
    return np.array(1.0 - total / (GROUP * B2), dtype=np.float32)


def kernel(outputs, targets):
    nc1, nc2 = _get_ncs()
    res1 = run_bass_kernel_spmd(nc1, make_in_maps1(outputs, targets),
                                core_ids=list(range(NCORES)))
    res2 = run_bass_kernel_spmd(nc2, make_in_maps2(res1.results),
                                core_ids=list(range(NCORES)))
    return finish(res2.results)


if __name__ == "__main__":
    import reference as ref
    inputs = ref.setup_inputs()
    actual = kernel(**{k: np.asarray(v) for k, v in inputs.items()})
    print("kernel result:", actual)


# revision 46
# speedup vs baseline: 1.0332x; 1.0332x over previous
"""APKDA loss (pool+normalize -> SmoothAP) as two distributed Bass launches on
8 TRN2 NeuronCores.

Math restructuring vs the reference (same as the earlier baseline):
  - Only the diagonal class-blocks of sim_all_rk are used, so per query q we
    need rank sums only over its 16 same-class columns j:
        r_all[q,j] = 1 + sum_k relu(S[q,k] - S[q,j])   (k over all 512)
        r_pos[q,j] = 1 + sum_k relu(Sg[q,k] - Sg[q,j]) (k over the 16-group)
  - L2-normalizing the hw-sum equals normalizing the hw-mean.
  - Column order of S is irrelevant; each core's keys are rotated so its own
    64 columns sit first.

Precision: inputs are cast to fp8_e4m3 on the host (4x fewer HBM bytes; the
errors average out over the 49-wide pooling and 512-d normalized dot
products; measured end-to-end rel-err ~1e-4 vs the f32 reference, tolerance
2e-2).  f is also shipped as fp8 in phase 2; biases stay f32.

Phase 1 (memory-bound): core m owns batch rows 32m..32m+31 of both branches.
  - Branch A (outputs) is pooled on the PE: host lays the shard out as 13
    h-plane tiles [128 part=(hh,b), 512 c] fp8; 13 accumulating matmuls
    against a one-hot [128, 32] "eye" (eye[p, p%32]=1) give
    psum[b, c] = sum_h x[b, c, h] exactly (f32 accumulate).
  - Branch B (targets) is reduced on the DVE from the classic
    [128 part=(g,b), (c h)] layout in 4 c-chunks (TensorReduce is ~1.1ns/col
    regardless of dtype; one branch = 7.1us; PE's 13 matmuls ~5.6us + the
    psum->sbuf copy are the other pole -- balanced).
  - The logical-core pair shares ~400GB/s of SDMA/HBM, so the 1.7MB shard
    lands over ~9us however it is laid out; sync carries xa, scalar xb,
    chunked so both engines chase the stream and the last chunks are small.
    Engine DMA issue costs ~0.6-1.1us per 128-descriptor dma_start, and
    per-partition runs below ~4KB lose DMA efficiency -- so few, fat,
    full-width chunks.  Out: pooled sums, bf16, split so the tail DMA is
    tiny.

Phase 2: host normalizes f, rotates keys per core, computes the own-class
Gram and rank biases in f32 (tiny), and ships fp8 fT (rhs [128p, 4x512] =
2KB/partition single DMA) + fp8 ccin (own queries duplicated on both psum
partition halves) + f32 biases.  PE computes the S slice (8 fp8 matmuls,
2 psum banks x 4 d-chunks); ACT/DVE stage S to bf16 and run the 8 big rank
ops (~730ns each, split 4/4) with accum_out; the 8 r_pos ops run early on
ACT straight from the host Gram.  Host applies +1/division/total.

Collectives/single-launch variants measured earlier: in-kernel AllGather
133.7us (nrt cc latency floor ~100us); remote-dma exchange requires
cross-core waits that absorb the profiler's multi-ms launch skew into core
0's measured time.  So the f exchange goes through the host (two NEFF
launches; each carries ~6us preamble + ~7us NEFF postamble of fixed cost).
"""

import numpy as np
import ml_dtypes

import concourse.bass as bass
import concourse.bacc as bacc
import concourse.mybir as mybir
from concourse.bass_utils import run_bass_kernel_spmd

F32 = mybir.dt.float32
BF16 = mybir.dt.bfloat16
F8 = mybir.dt.float8e4
NP_F8 = ml_dtypes.float8_e4m3
NCORES = 8
BATCH, FEAT, HW = 256, 512, 49
BPC = BATCH // NCORES          # 32 batch rows per branch per core
GROUP, B2 = 16, 512

XB_CHUNKS = [32, 40, 40, 16]   # branch-B c-units (x49 cols) per DVE chunk
XA_CHUNKS = [2, 4, 4, 2]       # branch-A h-plane tiles per PE chunk (12 full
                               # tiles; plane 48 ships separately as [32,512])


def _hoist_to_preamble(nc, head, early=True):
    """Move the given BassInstructions to the head of their engine's
    stream.  early=True inserts right after the engine's first DRAIN --
    BEFORE the ctor barrier that serializes everything behind gpsimd's
    ~6us library-load, so the transfers run during the dead preamble
    window.  early=False is the old post-preamble_end hoist."""
    entry = nc.main_func.blocks[0]
    lst = entry.instructions
    for bi in reversed(head):
        inst = bi.ins
        lst.remove(inst)
        if early:
            idx = next(i for i, x in enumerate(lst)
                       if getattr(x, "engine", None) == inst.engine
                       and isinstance(x, mybir.InstDrain))
            lst.insert(idx + 1, inst)
        else:
            pe_marker = nc.engines[inst.engine].preamble_end
            lst.insert(lst.index(pe_marker) + 1, inst)


def build_phase1(dbg=None):
    """fp8 pooling: PE eye-matmul for branch A, DVE reduce for branch B."""
    nc = bacc.Bacc("TRN2", target_bir_lowering=False, debug=False,
                   num_devices=NCORES)
    AX = mybir.AxisListType
    # xa: branch A h-plane tiles; [128, 6656] but only [0:32] of the last
    # 512 cols is read (plane 48).  xb: branch B [(g,b), (c h)] fp8.
    xa_d = nc.dram_tensor("xa", [128, 6656], F8, kind="ExternalInput")
    xb_d = nc.dram_tensor("xb", [128, 6272], F8, kind="ExternalInput")
    eye_d = nc.dram_tensor("eye", [128, 32], F8, kind="ExternalInput")
    pa_d = nc.dram_tensor("pa", [32, 512], BF16, kind="ExternalOutput")
    pt_d = nc.dram_tensor("pt", [128, 128], BF16, kind="ExternalOutput")

    xa = nc.alloc_sbuf_tensor("xa_s", [128, 6656], F8)
    xb = nc.alloc_sbuf_tensor("xb_s", [128, 6272], F8)
    eye = nc.alloc_sbuf_tensor("eye_s", [128, 32], F8)
    pa = nc.alloc_sbuf_tensor("pa_s", [32, 512], BF16)
    pt = nc.alloc_sbuf_tensor("pt_s", [128, 128], BF16)
    ps = nc.alloc_psum_tensor("ps", [32, 512], F32)

    sEYE = nc.alloc_semaphore("sEYE")
    sB = [nc.alloc_semaphore(f"sB{i}") for i in range(len(XB_CHUNKS))]
    sA = [nc.alloc_semaphore(f"sA{i}") for i in range(len(XA_CHUNKS))]
    sAL = nc.alloc_semaphore("sAL")
    sPE = nc.alloc_semaphore("sPE")
    sCP = nc.alloc_semaphore("sCP")
    sRED = nc.alloc_semaphore("sRED")
    sOUT = nc.alloc_semaphore("sOUT")
    sems = [sEYE] + sB + sA + [sAL, sPE, sCP, sRED, sOUT]
    nums = sorted(s.num for s in sems)
    assert nums == list(range(nums[0], nums[0] + len(sems))), nums
    sem_range = range(nums[0], nums[0] + len(sems))

    head = []
    # --- chunked DMAs on both queues: the logical-core pair shares
    # ~400GB/s of SDMA/HBM, so the shard lands over ~8.5us no matter the
    # layout; chunking lets PE/DVE chase the stream and finish ~0.5us
    # after the last byte instead of starting then. sync=xa, scalar=xb.
    # ALL input DMAs on the scalar queue in one explicit FIFO interleave:
    # scalar's queue issues ~0.8us earlier than sync's and its first
    # packet time is stable (sync's drifts 7.4-9.2us run to run), one
    # queue can carry the full per-core share, and FIFO makes the
    # engine-feed order deterministic.  xa's tail is sequenced before
    # xb's last chunk because the PE branch has the longer completion
    # chain (psum copy + out).
    xao = [0, 2, 6, 10]
    xbo = [0, 32, 72, 112]

    def dma_xa(i):
        t0, nt = xao[i], XA_CHUNKS[i]
        head.append(nc.scalar.dma_start(
            xa.ap()[:, 512 * t0:512 * (t0 + nt)],
            xa_d.ap()[:, 512 * t0:512 * (t0 + nt)]).then_inc(sA[i], 16))

    def dma_xb(i):
        c0, w = xbo[i], XB_CHUNKS[i]
        head.append(nc.scalar.dma_start(
            xb.ap()[:, 49 * c0:49 * (c0 + w)],
            xb_d.ap()[:, 49 * c0:49 * (c0 + w)]).then_inc(sB[i], 16))

    dma_xb(0)
    dma_xa(0)
    head.append(nc.scalar.dma_start(eye.ap(), eye_d.ap()).then_inc(sEYE, 16))
    dma_xb(1)
    dma_xa(1)
    dma_xb(2)
    dma_xa(2)
    dma_xa(3)
    head.append(nc.scalar.dma_start(
        xa.ap()[0:32, 6144:6656], xa_d.ap()[0:32, 6144:6656]
    ).then_inc(sAL, 16))
    dma_xb(3)

    # --- tensor: 12 full accumulating matmuls + the K=32 plane-48 one ----
    nc.tensor.wait_ge(sEYE, 16)
    t0 = 0
    for i, nt in enumerate(XA_CHUNKS):
        nc.tensor.wait_ge(sA[i], 16)
        for t in range(t0, t0 + nt):
            nc.tensor.matmul(ps.ap(), eye.ap(),
                             xa.ap()[:, 512 * t:512 * (t + 1)],
                             start=(t == 0), stop=False)
        t0 += nt
    nc.tensor.wait_ge(sAL, 16)
    mm = nc.tensor.matmul(ps.ap(), eye.ap()[0:32, :],
                          xa.ap()[0:32, 6144:6656], start=False, stop=True)
    mm.then_inc(sPE, 1)

    # --- scalar (ACT): psum -> bf16 pa, then out DMA ---------------------
    nc.scalar.wait_ge(sPE, 1)
    with nc.allow_low_precision("pooled sums, single bf16 round"):
        nc.scalar.copy(pa.ap(), ps.ap()).then_inc(sCP, 1)
    nc.scalar.wait_ge(sCP, 1)
    nc.scalar.dma_start(pa_d.ap(), pa.ap()).then_inc(sOUT, 16)

    # --- vector (DVE): 4 reduces chasing the xb chunks -------------------
    c0 = 0
    nw = len(XB_CHUNKS)
    for i, w in enumerate(XB_CHUNKS):
        nc.vector.wait_ge(sB[i], 16)
        with nc.allow_low_precision("f32 accumulate, single bf16 round"):
            red = nc.vector.reduce_sum(
                pt.ap()[:, c0:c0 + w],
                xb.ap()[:, 49 * c0:49 * (c0 + w)].rearrange(
                    "p (c h) -> p c h", h=HW),
                axis=AX.X)
        if i >= nw - 2:
            red.then_inc(sRED, 1)
        c0 += w
    # split pt out: everything but the last chunk ships early, so the
    # final out DMA (and its HBM completion) is tiny
    c1 = sum(XB_CHUNKS[:-1])
    nc.sync.wait_ge(sRED, 1)
    nc.sync.dma_start(pt_d.ap()[:, 0:c1], pt.ap()[:, 0:c1]).then_inc(sOUT, 16)
    nc.sync.wait_ge(sRED, 2)
    nc.sync.dma_start(pt_d.ap()[:, c1:128], pt.ap()[:, c1:128]).then_inc(sOUT, 16)

    # --- gpsimd: restore sems, halt --------------------------------------
    nc.gpsimd.wait_ge(sOUT, 48)
    nc.gpsimd.dma_reset(sem_range)
    nc.gpsimd.sem_clear(sem_range)

    _hoist_to_preamble(nc, head)
    nc.compile()
    return nc


def build_phase2(dbg=None):
    """S slice + rank sums from fp8 fT (own cols at keys 0..63).
    in: rhs [128, 2048] fp8 (4 d-blocks side by side, rhs[p, 512g+k] =
    fT[d=128g+p, key k]); ccin [128, 512] fp8 (ccin[p, 128g+64*two+q] =
    f_own[q, d=128g+p], queries duplicated on the two psum halves);
    bias [128, 24] f32 (cols 0-15 SgD, 16-23 B8 = -bias per j-slot);
    out racc [128, 16] f32 (cols 0-7 raw r_all, 8-15 raw r_pos)."""
    nc = bacc.Bacc("TRN2", target_bir_lowering=False, debug=False,
                   num_devices=NCORES)
    AF = mybir.ActivationFunctionType
    ALU = mybir.AluOpType
    rhs_d = nc.dram_tensor("rhs", [128, 2048], F8, kind="ExternalInput")
    ccin_d = nc.dram_tensor("ccin", [128, 512], F8, kind="ExternalInput")
    bias_d = nc.dram_tensor("bias", [128, 24], F32, kind="ExternalInput")
    out_d = nc.dram_tensor("out", [128, 16], F32, kind="ExternalOutput")

    rhs = nc.alloc_sbuf_tensor("rhs_s", [128, 2048], F8)
    ccin = nc.alloc_sbuf_tensor("ccin_s", [128, 512], F8)
    bias = nc.alloc_sbuf_tensor("bias_s", [128, 24], F32)
    Sb = nc.alloc_sbuf_tensor("Sb", [128, 512], BF16)
    scrap_v = nc.alloc_sbuf_tensor("scrap_v", [128, 512], BF16)
    scrap_a = nc.alloc_sbuf_tensor("scrap_a", [128, 512], BF16)
    scrap_s = nc.alloc_sbuf_tensor("scrap_s", [128, 16], F32)
    zeros = nc.alloc_sbuf_tensor("zeros", [128, 512], BF16)
    warm = nc.alloc_sbuf_tensor("warm", [128, 1], F32)
    racc = nc.alloc_sbuf_tensor("racc", [128, 16], F32)
    ps0 = nc.alloc_psum_tensor("ps0", [128, 256], F32)
    ps1 = nc.alloc_psum_tensor("ps1", [128, 256], F32)

    sRHg = [nc.alloc_semaphore(f"sRH{g}") for g in range(4)]
    sCC = nc.alloc_semaphore("sCC")
    sBI = nc.alloc_semaphore("sBI")
    sPE0 = nc.alloc_semaphore("sPE0")
    sPE1 = nc.alloc_semaphore("sPE1")
    sST = nc.alloc_semaphore("sST")
    sRKS = nc.alloc_semaphore("sRKS")
    sRKB = nc.alloc_semaphore("sRKB")
    sRKA = nc.alloc_semaphore("sRKA")
    sOUT = nc.alloc_semaphore("sOUT")
    sems = sRHg + [sCC, sBI, sPE0, sPE1, sST, sRKS, sRKB, sRKA, sOUT]
    nums = sorted(s.num for s in sems)
    assert nums == list(range(nums[0], nums[0] + len(sems))), nums
    sem_range = range(nums[0], nums[0] + len(sems))

    head = []
    head.append(nc.sync.dma_start(
        rhs.ap()[:, 0:1024], rhs_d.ap()[:, 0:1024]).then_inc(sRHg[0], 16))
    head.append(nc.scalar.dma_start(bias.ap(), bias_d.ap()).then_inc(sBI, 16))
    head.append(nc.scalar.dma_start(ccin.ap(), ccin_d.ap()).then_inc(sCC, 16))
    head.append(nc.scalar.dma_start(
        rhs.ap()[:, 1024:2048], rhs_d.ap()[:, 1024:2048]).then_inc(sRHg[2], 16))

    # --- tensor: 8 fp8 matmuls chasing the rhs halves --------------------
    nc.tensor.wait_ge(sCC, 16)
    for g in range(4):
        if g % 2 == 0:
            nc.tensor.wait_ge(sRHg[g], 16)
        mm0 = nc.tensor.matmul(ps0.ap(), ccin.ap()[:, 128 * g:128 * (g + 1)],
                               rhs.ap()[:, 512 * g:512 * g + 256],
                               start=(g == 0), stop=(g == 3))
        mm1 = nc.tensor.matmul(ps1.ap(), ccin.ap()[:, 128 * g:128 * (g + 1)],
                               rhs.ap()[:, 512 * g + 256:512 * (g + 1)],
                               start=(g == 0), stop=(g == 3))
        if g == 3:
            mm0.then_inc(sPE0, 1)
            mm1.then_inc(sPE1, 1)

    # --- scalar (ACT): warm, ps0 stage, 4 big ranks ----------------------
    nc.scalar.wait_ge(sBI, 16)
    nc.scalar.activation(warm.ap(), bias.ap()[:, 0:1], AF.Relu)
    nc.scalar.wait_ge(sPE0, 1)
    with nc.allow_low_precision("psum f32 -> bf16 S"):
        nc.scalar.copy(Sb.ap()[:, 0:256], ps0.ap()).then_inc(sST, 1)
    nc.scalar.wait_ge(sST, 2)
    with nc.allow_low_precision("bf16 rank scrap, f32 accum"):
        for i in range(4, 8):
            nc.scalar.activation(
                scrap_a.ap(), Sb.ap(), AF.Relu,
                bias=bias.ap()[:, 16 + i:17 + i],
                accum_out=racc.ap()[:, i:i + 1]).then_inc(sRKA, 1)

    # --- vector (DVE): early r_pos ranks (cheap 85ns accum reads), ps1
    # stage, 4 big ranks --------------------------------------------------
    nc.vector.memset(zeros.ap(), 0.0)
    nc.vector.wait_ge(sBI, 16)
    for i in range(8):
        nc.vector.scalar_tensor_tensor(
            out=scrap_s.ap(), in0=bias.ap()[:, 0:16],
            scalar=bias.ap()[:, 16 + i:17 + i], in1=zeros.ap()[:, 0:16],
            op0=ALU.add, op1=ALU.max,
            accum_out=racc.ap()[:, 8 + i:9 + i]).then_inc(sRKS, 1)
    nc.vector.wait_ge(sPE1, 1)
    with nc.allow_low_precision("psum f32 -> bf16 S"):
        nc.vector.tensor_copy(Sb.ap()[:, 256:512], ps1.ap()).then_inc(sST, 1)
    nc.vector.wait_ge(sST, 2)
    with nc.allow_low_precision("bf16 rank scrap, f32 accum"):
        for i in range(4):
            nc.vector.scalar_tensor_tensor(
                out=scrap_v.ap(), in0=Sb.ap(),
                scalar=bias.ap()[:, 16 + i:17 + i], in1=zeros.ap(),
                op0=ALU.add, op1=ALU.max,
                accum_out=racc.ap()[:, i:i + 1]).then_inc(sRKB, 1)

    # --- sync: split out DMAs (r_pos half and each engine's big half ship
    # as soon as they are ready, so the last HBM completion starts early) -
    nc.sync.wait_ge(sRKS, 8)
    nc.sync.dma_start(out_d.ap()[:, 8:16], racc.ap()[:, 8:16]).then_inc(sOUT, 16)
    nc.sync.wait_ge(sRKB, 4)
    nc.sync.dma_start(out_d.ap()[:, 0:4], racc.ap()[:, 0:4]).then_inc(sOUT, 16)
    nc.sync.wait_ge(sRKA, 4)
    nc.sync.dma_start(out_d.ap()[:, 4:8], racc.ap()[:, 4:8]).then_inc(sOUT, 16)

    # --- gpsimd: restore sems, halt --------------------------------------
    nc.gpsimd.wait_ge(sOUT, 48)
    nc.gpsimd.dma_reset(sem_range)
    nc.gpsimd.sem_clear(sem_range)

    _hoist_to_preamble(nc, head)
    nc.compile()
    return nc


_NC1 = None
_NC2 = None


def _get_ncs():
    global _NC1, _NC2
    if _NC1 is None:
        _NC1 = build_phase1()
        _NC2 = build_phase2()
    return _NC1, _NC2


# one-hot pooling matrix: eye[p, p%32] = 1
_EYE = np.zeros((128, 32), np.float32)
_EYE[np.arange(128), np.arange(128) % 32] = 1.0
_EYE = _EYE.astype(NP_F8)

# column permutation: branch-ordered [out b, tgt b] -> reference interleaved
# col = 16*(b//8) + 8*branch + b%8
_PERM = np.empty(64, np.int64)
for _b in range(32):
    _PERM[16 * (_b // 8) + (_b % 8)] = _b            # outputs branch
    _PERM[16 * (_b // 8) + 8 + (_b % 8)] = 32 + _b   # targets branch


def make_in_maps1(outputs, targets):
    outputs = np.asarray(outputs, dtype=np.float32).reshape(BATCH, FEAT, HW)
    targets = np.asarray(targets, dtype=np.float32).reshape(BATCH, FEAT, HW)
    o8 = outputs.astype(NP_F8)
    t8 = targets.astype(NP_F8)
    maps = []
    for m in range(NCORES):
        o = o8[m * BPC:(m + 1) * BPC]                  # [32, 512, 49]
        t = t8[m * BPC:(m + 1) * BPC]
        # branch A: h-plane tiles.  xa[32*hh+b, 512*t+c] = o[b, c, 4t+hh]
        ot = o.transpose(2, 0, 1)                      # [49, 32, 512]
        xa = np.zeros((128, 6656), NP_F8)
        xa[:, 0:6144] = (ot[0:48].reshape(12, 4, BPC, FEAT)
                         .transpose(0, 1, 2, 3)        # [t, hh, b, c]
                         .reshape(12, 128, FEAT)
                         .transpose(1, 0, 2)           # [p, t, c]
                         .reshape(128, 6144))
        xa[0:32, 6144:6656] = ot[48]                   # plane 48
        # branch B: [(g,b), (c h)]: xb[32g+b] = t[b, 128g:128g+128, :].flat
        xb = (t.reshape(BPC, 4, 128, HW)
              .transpose(1, 0, 2, 3)                   # [g, b, c_local, h]
              .reshape(128, 6272))
        maps.append({"xa": np.ascontiguousarray(xa),
                     "xb": np.ascontiguousarray(xb), "eye": _EYE})
    return maps


def make_in_maps2(results1):
    """pooled sums -> normalized f, per-core rotated fp8 fT + biases."""
    blocks = []   # per core: f rows [64, 512] f32 in reference order
    for m in range(NCORES):
        pa = results1[m]["pa"].astype(np.float32)      # [32, 512] b-major
        pt = results1[m]["pt"].astype(np.float32)      # [128(g,b), 128]
        vt = np.concatenate([pt[32 * g:32 * (g + 1), :] for g in range(4)],
                            axis=1)                    # [32, 512]
        f = np.concatenate([pa, vt], axis=0)           # [64, 512]
        f /= np.linalg.norm(f, axis=1, keepdims=True)
        blocks.append(f[_PERM, :])                     # reference order
    f8 = [b.astype(NP_F8) for b in blocks]
    f8f = [b.astype(np.float32) for b in f8]           # fp8-quantized f32
    maps = []
    for m in range(NCORES):
        # keys rotated: own 64 first
        rot = np.concatenate([f8f[(m + j) % NCORES] for j in range(NCORES)],
                             axis=0)                   # [512 keys, 512 d]
        rhs = np.ascontiguousarray(
            rot.T.reshape(4, 128, 512).transpose(1, 0, 2).reshape(128, 2048)
        ).astype(NP_F8)                                # [p, 512g+k]
        own = f8f[m]                                   # [64 q, 512 d]
        ccin = np.empty((128, 512), np.float32)
        for g in range(4):
            blk = own[:, 128 * g:128 * (g + 1)].T      # [128 d, 64 q]
            ccin[:, 128 * g + 0:128 * g + 64] = blk
            ccin[:, 128 * g + 64:128 * g + 128] = blk
        sg = own @ own.T                               # [64, 64] f32
        # SgD[p, j] = Sg[qi, 16*(qi//16)+j], qi = p % 64
        qi = np.arange(64)
        base = (qi // 16) * 16
        sgd64 = sg[qi[:, None], base[:, None] + np.arange(16)[None]]  # [64,16]
        sgd = np.concatenate([sgd64, sgd64], axis=0)   # [128, 16]
        b8 = np.empty((128, 8), np.float32)
        b8[0:64] = -sgd64[:, 0:8]
        b8[64:128] = -sgd64[:, 8:16]
        biasm = np.concatenate([sgd, b8], axis=1)      # [128, 24]
        maps.append({"rhs": rhs, "ccin": ccin.astype(NP_F8),
                     "bias": np.ascontiguousarray(biasm)})
    return maps


def finish(results2):
    total = 0.0
    for m in range(NCORES):
        racc = results2[m]["out"].astype(np.float64)   # [128, 16]
        total += ((1.0 + racc[:, 8:16]) / (1.0 + racc[:, 0:8])).sum()
    return np.array(1.0 - total / (GROUP * B2), dtype=np.float32)


def kernel(outputs, targets):
    nc1, nc2 = _get_ncs()
    res1 = run_bass_kernel_spmd(nc1, make_in_maps1(outputs, targets),
                                core_ids=list(range(NCORES)))
    res2 = run_bass_kernel_spmd(nc2, make_in_maps2(res1.results),
                                core_ids=list(range(NCORES)))
    return finish(res2.results)


if __name__ == "__main__":
    import reference as ref
    inputs = ref.setup_inputs()
    actual = kernel(**{k: np.asarray(v) for k, v in inputs.items()})
    print("kernel result:", actual)
